# revision 1
# baseline (speedup 1.0000x reference)
"""TRN2 Bass kernel for nn_Cvx_KnapsackNet (MLP + 200-iter ADMM projection QP).

Math: the reference ADMM iteration
    v   = w + rho*(z - u)
    rhs = v @ A.T - (1+rho)*b
    y   = cho_solve(A A^T, rhs.T).T
    x   = (v - y @ A) / (1+rho)
    z   = max(x+u, 0);  u = u + x - z
collapses algebraically. With P' = (I - A^T (A A^T)^{-1} A)/(1+rho),
c = b @ (A A^T)^{-1} A, and state q_k = x_k + u_{k-1}:
    t_k     = w + |q_k|          (t_0 = w)
    x_k     = t_k @ P' + c
    q_{k+1} = x_k + min(q_k, 0)
One [B,1152]x[1152,1152] matmul plus two fused elementwise ops per
iteration. c is folded into the matmul via an extra "ones" row (row
1030 of the padded state is held at 1; row 1030 of P' holds c).

The map contracts at ~0.64/iter and reaches the fp32 noise floor by
~iter 40 (the reference runs 200 converged iterations), so a truncated
schedule reproduces x_200 to ~1e-6 relative error. WARM iterations run
with bf16 operands (4x faster PE) to converge the active set cheaply;
POLISH iterations run exact fp32 and contract the bf16-induced error
(~5e-4) below 2e-6. The polish phase converges to the exact fp32 fixed
point, so final accuracy matches the pure-fp32 kernel.

Sharding: pure data parallel, batch 1024 -> 128 rows per NeuronCore.
On-chip layout is transposed ([n2p=1152 rows, 128 batch cols], 9 tiles
of 128 partitions) so the matmul contraction runs over partitions.
"""
import sys
sys.path.insert(0, '/opt/trn_rl_repo')
import os
import numpy as np

B, C, H, R, K = 1024, 32, 3200, 500, 30
RHO = 1.0
N1 = K + R              # 530
N2 = R + K + R          # 1030
N2P = 1152              # 9 * 128
NT = N2P // 128         # 9 state tiles
BIAS_ROW = N2           # 1030
NCORES = 8
BL = B // NCORES        # 128 batch rows per core
HT = H // 128           # 25 hidden tiles
WARM = int(os.environ.get("KNAP_WARM", "26"))
POLISH = int(os.environ.get("KNAP_POLISH", "14"))
MC_W = 5                # m-tiles per W2 chunk
N_MC = HT // MC_W       # 5 chunks
CT = 512 // 128         # 4 cost tiles (500 padded to 512)

_CACHE = {}


def _host_precompute(W1, b1, W2, b2, W3, b3, weights_mat, capacities):
    """float64 host math -> packed fp32/bf16 device constants."""
    import ml_dtypes
    wm = weights_mat.astype(np.float64)
    cap = capacities.astype(np.float64)
    A = np.zeros((N1, N2), np.float64)
    A[:K, :R] = wm
    A[:K, R:R + K] = np.eye(K)
    A[K:, :R] = np.eye(R)
    A[K:, R + K:] = np.eye(R)
    b = np.concatenate([cap, np.ones(R)])
    M = np.linalg.inv(A @ A.T)
    P = (np.eye(N2) - A.T @ M @ A) / (1.0 + RHO)
    c = b @ M @ A
    Pbig = np.zeros((N2P, N2P), np.float32)
    Pbig[:N2, :N2] = P.astype(np.float32)
    Pbig[BIAS_ROW, :N2] = c.astype(np.float32)
    # partition-major blocked: PbigPM[p, (k*NT+j)*128 + f] = Pbig[k*128+p, j*128+f]
    PbigPM = np.ascontiguousarray(
        Pbig.reshape(NT, 128, NT, 128).transpose(1, 0, 2, 3).reshape(128, NT * NT * 128))
    PbigBF = PbigPM.astype(ml_dtypes.bfloat16)

    W3p = np.zeros((512, H), np.float32)
    W3p[:R] = W3
    # w3PM[p, k*512 + f] = W3p.T[k*128+p, f]
    w3PM = np.ascontiguousarray(
        W3p.T.reshape(HT, 128, 512).transpose(1, 0, 2).reshape(128, HT * 512))

    b1R = np.ascontiguousarray(b1.reshape(HT, 128).T)       # [128, 25]
    b2R = np.ascontiguousarray(b2.reshape(HT, 128).T)       # [128, 25]
    b3p = np.zeros(512, np.float32)
    b3p[:R] = b3
    b3R = np.ascontiguousarray(b3p.reshape(CT, 128).T)      # [128, 4]
    # padding tiles 4..8 of w (zeros; bias-row 1030 -> tile 8, partition 6 = 1)
    wpad = np.zeros((128, (NT - CT) * 128), np.float32)
    wpad[BIAS_ROW - 8 * 128, (8 - CT) * 128:(9 - CT) * 128] = 1.0

    small = np.concatenate([b1R, b2R, b3R, wpad], axis=1).astype(np.float32)
    pack = PbigPM
    W1T = np.ascontiguousarray(W1.T)                        # [32, 3200]
    W2T = np.ascontiguousarray(W2.T)                        # [3200, 3200]
    return pack, small, PbigBF, w3PM, W1T, W2T


def _build_nc():
    import concourse.bacc as bacc
    import concourse.mybir as mybir
    from concourse import tile
    from concourse.tile_rust import add_dep_helper

    f32 = mybir.dt.float32
    bf16 = mybir.dt.bfloat16
    SMALL_W = HT + HT + CT + (NT - CT) * 128
    OFF_B1 = 0
    OFF_B2 = OFF_B1 + HT
    OFF_B3 = OFF_B2 + HT
    OFF_WP = OFF_B3 + CT

    nc = bacc.Bacc("TRN2", target_bir_lowering=False, debug=False, num_devices=NCORES)
    small_d = nc.dram_tensor("small_d", [128, SMALL_W], f32, kind="ExternalInput").ap()
    pack_d = nc.dram_tensor("pack_d", [128, NT * NT * 128], f32, kind="ExternalInput").ap()
    pbf_d = nc.dram_tensor("pbf_d", [128, NT * NT * 128], bf16, kind="ExternalInput").ap()
    w3_d = nc.dram_tensor("w3_d", [128, HT * 512], f32, kind="ExternalInput").ap()
    dw_d = nc.dram_tensor("dw_d", [C, BL + H], f32, kind="ExternalInput").ap()
    w2t_d = nc.dram_tensor("w2t_d", [H, H], f32, kind="ExternalInput").ap()
    out_d = nc.dram_tensor("out_d", [128, N2P], f32, kind="ExternalOutput").ap()

    Act = mybir.ActivationFunctionType
    Alu = mybir.AluOpType
    TOTAL = WARM + POLISH

    with tile.TileContext(nc) as tc:
        with tc.tile_pool(name="sb", bufs=1) as sb, \
             tc.tile_pool(name="wst", bufs=4) as wst, \
             tc.tile_pool(name="mlp", bufs=1) as mlp, \
             tc.tile_pool(name="ps", bufs=8, space="PSUM") as pspool:
            dw = mlp.tile([C, BL + H], f32)
            nc.sync.dma_start(out=dw[:], in_=dw_d[:])
            sm = sb.tile([128, SMALL_W], f32)
            nc.sync.dma_start(out=sm[:], in_=small_d[:])
            pbf = sb.tile([128, NT * NT * 128], bf16)
            pk = sb.tile([128, NT * NT * 128], f32)

            P_sb = pk[:, :]
            b1R = sm[:, OFF_B1:OFF_B1 + HT]
            b2R = sm[:, OFF_B2:OFF_B2 + HT]
            b3R = sm[:, OFF_B3:OFF_B3 + CT]
            dT = dw[:, 0:BL]
            w1T = dw[:, BL:BL + H]

            h1 = mlp.tile([128, HT * 128], f32)   # h1T tiles: [p, m*128+b]
            h2 = mlp.tile([128, HT * 128], f32)
            w_sb = sb.tile([128, N2P], f32)       # wT tiles: [p, j*128+b]
            wb_sb = sb.tile([128, N2P], bf16)
            q_sb = sb.tile([128, N2P], f32)
            a_sb = sb.tile([128, N2P], f32)
            t_bufs = [sb.tile([128, N2P], f32, name=f"t{i}") for i in range(3)]
            tb_bufs = [sb.tile([128, N2P], bf16, name=f"tb{i}") for i in range(3)]
            out_sb = sb.tile([128, N2P], f32)

            nc.vector.memset(q_sb[:], 0.0)
            nc.vector.tensor_copy(w_sb[:, CT * 128:],
                                  sm[:, OFF_WP:OFF_WP + (NT - CT) * 128])

            # ---- MLP layer 1: h1T[m] = prelu(W1T[:,m].T @ dT + b1, 0.1) ----
            for m in range(HT):
                ps_t = pspool.tile([128, 128], f32, tag="ps", name="ps_t")
                nc.tensor.matmul(ps_t[:], w1T[:, m * 128:(m + 1) * 128], dT,
                                 start=True, stop=True)
                nc.scalar.activation(h1[:, m * 128:(m + 1) * 128], ps_t[:],
                                     Act.Prelu, bias=b1R[:, m:m + 1], alpha=0.1)

            # ---- MLP layer 2: stream W2T; h2T[m] = prelu(sum_k W2T[k,m].T @ h1T[k] + b2) ----
            mark_mid = None
            for mc in range(N_MC):
                ps_list = [pspool.tile([128, 128], f32, tag="ps", name="ps_t")
                           for _ in range(MC_W)]
                for k in range(HT):
                    w2blk = wst.tile([128, MC_W * 128], f32, name="w2blk")
                    nc.sync.dma_start(
                        out=w2blk[:],
                        in_=w2t_d[k * 128:(k + 1) * 128,
                                  mc * MC_W * 128:(mc + 1) * MC_W * 128])
                    for mi in range(MC_W):
                        mm = nc.tensor.matmul(ps_list[mi][:],
                                         w2blk[:, mi * 128:(mi + 1) * 128],
                                         h1[:, k * 128:(k + 1) * 128],
                                         start=(k == 0), stop=(k == HT - 1))
                        if mc == 2 and k == 0 and mi == 0:
                            mark_mid = mm.ins
                for mi in range(MC_W):
                    m = mc * MC_W + mi
                    nc.scalar.activation(h2[:, m * 128:(m + 1) * 128], ps_list[mi][:],
                                         Act.Prelu, bias=b2R[:, m:m + 1], alpha=0.1)

            # ---- cost layer: w tiles 0..3 = sum_k W3p.T[k] @ h2T[k] + b3 (w3 streamed) ----
            ps_cost = [pspool.tile([128, 128], f32, tag="ps", name="ps_t")
                       for _ in range(CT)]
            for k in range(HT):
                w3blk = wst.tile([128, 512], f32, name="w3blk", bufs=3)
                nc.sync.dma_start(out=w3blk[:], in_=w3_d[:, k * 512:(k + 1) * 512])
                for m in range(CT):
                    nc.tensor.matmul(ps_cost[m][:],
                                     w3blk[:, m * 128:(m + 1) * 128],
                                     h2[:, k * 128:(k + 1) * 128],
                                     start=(k == 0), stop=(k == HT - 1))
            for m in range(CT):
                nc.scalar.activation(w_sb[:, m * 128:(m + 1) * 128], ps_cost[m][:],
                                     Act.Identity, bias=b3R[:, m:m + 1])
            nc.vector.tensor_copy(wb_sb[:], w_sb[:])

            pbf_dma = nc.sync.dma_start(out=pbf[:], in_=pbf_d[:])
            add_dep_helper(pbf_dma.ins, mark_mid, sync=True,
                           reason="defer bf16 P load past MLP DMA crunch")

            # ---- ADMM iterations: WARM in bf16, POLISH in fp32 ----
            for it in range(TOTAL):
                last = (it == TOTAL - 1)
                use_bf = it < WARM
                if it == 0:
                    cur = wb_sb if WARM > 0 else w_sb
                elif use_bf:
                    cur = tb_bufs[(it - 1) % 3]
                else:
                    cur = t_bufs[(it - 1) % 3]
                Pmat = pbf if use_bf else P_sb
                for j in range(NT):
                    ps_t = pspool.tile([128, 128], f32, tag="ps", name="ps_t")
                    for k in range(NT):
                        mm = nc.tensor.matmul(ps_t[:],
                                         Pmat[:, (k * NT + j) * 128:(k * NT + j + 1) * 128],
                                         cur[:, k * 128:(k + 1) * 128],
                                         start=(k == 0), stop=(k == NT - 1))
                        if it == 0 and j == 0 and k == 0:
                            pack_dma = nc.sync.dma_start(out=pk[:], in_=pack_d[:])
                            add_dep_helper(pack_dma.ins, mm.ins, sync=True,
                                           reason="defer fp32 P load to warm phase")
                    jj = slice(j * 128, (j + 1) * 128)
                    if last:
                        nc.scalar.activation(out_sb[:, jj], ps_t[:], Act.Copy)
                        nc.sync.dma_start(out=out_d[:, jj], in_=out_sb[:, jj])
                    else:
                        # q = min(q, 0) + x
                        nc.vector.scalar_tensor_tensor(
                            out=q_sb[:, jj], in0=q_sb[:, jj], scalar=0.0,
                            in1=ps_t[:], op0=Alu.min, op1=Alu.add)
                        nc.scalar.activation(a_sb[:, jj], q_sb[:, jj], Act.Abs)
                        if it < WARM - 1:
                            nc.vector.tensor_tensor(out=tb_bufs[it % 3][:, jj],
                                                    in0=a_sb[:, jj], in1=w_sb[:, jj],
                                                    op=Alu.add)
                        else:
                            nc.vector.tensor_tensor(out=t_bufs[it % 3][:, jj],
                                                    in0=a_sb[:, jj], in1=w_sb[:, jj],
                                                    op=Alu.add)


    nc.compile()
    return nc


def kernel(d, W1, b1, W2, b2, W3, b3, weights_mat, capacities):
    from concourse.bass_utils import run_bass_kernel_spmd

    d = np.asarray(d, np.float32)
    pack, small, PbigBF, w3PM, W1T, W2T = _host_precompute(
        np.asarray(W1, np.float32), np.asarray(b1, np.float32),
        np.asarray(W2, np.float32), np.asarray(b2, np.float32),
        np.asarray(W3, np.float32), np.asarray(b3, np.float32),
        np.asarray(weights_mat, np.float32), np.asarray(capacities, np.float32))

    if "nc" not in _CACHE:
        _CACHE["nc"] = _build_nc()
    nc = _CACHE["nc"]

    in_maps = []
    for i in range(NCORES):
        dTc = np.ascontiguousarray(d[i * BL:(i + 1) * BL].T)      # [32, 128]
        dwc = np.concatenate([dTc, W1T], axis=1)                  # [32, 128+3200]
        in_maps.append({"pack_d": pack, "small_d": small, "pbf_d": PbigBF,
                        "w3_d": w3PM, "dw_d": dwc, "w2t_d": W2T})

    trace = bool(int(os.environ.get("KNAP_TRACE", "0")))
    res = run_bass_kernel_spmd(nc, in_maps, core_ids=list(range(NCORES)),
                               trace=trace)
    if trace:
        _CACHE["exec_time_ns"] = res.exec_time_ns
        _CACHE["trace"] = res.instructions_and_trace

    out = np.empty((B, N2), np.float32)
    for i in range(NCORES):
        arr = res.results[i]["out_d"]                              # [128, 1152]
        xc = arr.reshape(128, NT, 128).transpose(2, 1, 0).reshape(BL, N2P)
        out[i * BL:(i + 1) * BL] = xc[:, :N2]
    return out



# revision 2
# speedup vs baseline: 2.3116x; 2.3116x over previous
"""TRN2 Bass kernel for nn_Cvx_KnapsackNet (MLP + truncated ADMM projection QP).

Math: the reference ADMM iteration
    v   = w + rho*(z - u)
    rhs = v @ A.T - (1+rho)*b
    y   = cho_solve(A A^T, rhs.T).T
    x   = (v - y @ A) / (1+rho)
    z   = max(x+u, 0);  u = u + x - z
collapses algebraically. With P' = (I - A^T (A A^T)^{-1} A)/(1+rho),
c = b @ (A A^T)^{-1} A, and state q_k = x_k + u_{k-1}:
    t_k     = w + |q_k|          (t_0 = w)
    x_k     = t_k @ P' + c
    q_{k+1} = x_k + min(q_k, 0)
One [B,1152]x[1152,1152] matmul plus two fused elementwise ops per
iteration. c is folded into the matmul via an extra "ones" row (row
1030 of the padded state is held at 1; row 1030 of P' holds c).

The map contracts at ~0.64/iter; the harness gate is rel_err < 2e-2,
so a fully-bf16 pipeline (MLP weights/activations and ADMM operands in
bf16, fp32 PSUM accumulate) with 16 iterations reaches ~1.9e-3 —
dominated by the bf16 MLP cost-vector error, with a 10x margin.
fp32 matmuls run at half rate with unhidden LDWEIGHTS (~4x slower in
practice), so everything streams through the PE in bf16.

Sharding: pure data parallel, batch 1024 -> 128 rows per NeuronCore.
On-chip layout is transposed ([n2p=1152 rows, 128 batch cols], 9 tiles
of 128 partitions) so the matmul contraction runs over partitions.
"""
import sys
sys.path.insert(0, '/opt/trn_rl_repo')
import os
import numpy as np

B, C, H, R, K = 1024, 32, 3200, 500, 30
RHO = 1.0
N1 = K + R              # 530
N2 = R + K + R          # 1030
N2P = 1152              # 9 * 128
NT = N2P // 128         # 9 state tiles
BIAS_ROW = N2           # 1030
NCORES = 8
BL = B // NCORES        # 128 batch rows per core
HT = H // 128           # 25 hidden tiles
WARM = int(os.environ.get("KNAP_WARM", "16"))
MC_W = 5                # m-tiles per W2 chunk
N_MC = HT // MC_W       # 5 chunks
CT = 512 // 128         # 4 cost tiles (500 padded to 512)

_CACHE = {}


def _host_precompute(W1, b1, W2, b2, W3, b3, weights_mat, capacities):
    """float64 host math -> packed bf16/fp32 device constants."""
    import ml_dtypes
    bf = ml_dtypes.bfloat16
    wm = weights_mat.astype(np.float64)
    cap = capacities.astype(np.float64)
    A = np.zeros((N1, N2), np.float64)
    A[:K, :R] = wm
    A[:K, R:R + K] = np.eye(K)
    A[K:, :R] = np.eye(R)
    A[K:, R + K:] = np.eye(R)
    b = np.concatenate([cap, np.ones(R)])
    M = np.linalg.inv(A @ A.T)
    P = (np.eye(N2) - A.T @ M @ A) / (1.0 + RHO)
    c = b @ M @ A
    Pbig = np.zeros((N2P, N2P), np.float32)
    Pbig[:N2, :N2] = P.astype(np.float32)
    Pbig[BIAS_ROW, :N2] = c.astype(np.float32)
    # partition-major blocked: PbigPM[p, (k*NT+j)*128 + f] = Pbig[k*128+p, j*128+f]
    PbigPM = np.ascontiguousarray(
        Pbig.reshape(NT, 128, NT, 128).transpose(1, 0, 2, 3).reshape(128, NT * NT * 128))
    PbigBF = PbigPM.astype(bf)

    W3p = np.zeros((512, H), np.float32)
    W3p[:R] = W3
    # w3PM[p, k*512 + f] = W3p.T[k*128+p, f]
    w3PM = np.ascontiguousarray(
        W3p.T.reshape(HT, 128, 512).transpose(1, 0, 2).reshape(128, HT * 512)).astype(bf)

    b1R = np.ascontiguousarray(b1.reshape(HT, 128).T)       # [128, 25]
    b2R = np.ascontiguousarray(b2.reshape(HT, 128).T)       # [128, 25]
    b3p = np.zeros(512, np.float32)
    b3p[:R] = b3
    b3R = np.ascontiguousarray(b3p.reshape(CT, 128).T)      # [128, 4]
    # padding tiles 4..8 of w (zeros; bias-row 1030 -> tile 8, partition 6 = 1)
    wpad = np.zeros((128, (NT - CT) * 128), np.float32)
    wpad[BIAS_ROW - 8 * 128, (8 - CT) * 128:(9 - CT) * 128] = 1.0

    small = np.concatenate([b1R, b2R, b3R, wpad], axis=1).astype(np.float32)
    W1T = np.ascontiguousarray(W1.T).astype(bf)             # [32, 3200]
    W2T = np.ascontiguousarray(W2.T).astype(bf)             # [3200, 3200]
    return small, PbigBF, w3PM, W1T, W2T


def _build_nc():
    import concourse.bacc as bacc
    import concourse.mybir as mybir
    from concourse import tile
    from concourse.tile_rust import add_dep_helper

    f32 = mybir.dt.float32
    bf16 = mybir.dt.bfloat16
    SMALL_W = HT + HT + CT + (NT - CT) * 128
    OFF_B1 = 0
    OFF_B2 = OFF_B1 + HT
    OFF_B3 = OFF_B2 + HT
    OFF_WP = OFF_B3 + CT

    nc = bacc.Bacc("TRN2", target_bir_lowering=False, debug=False, num_devices=NCORES)
    small_d = nc.dram_tensor("small_d", [128, SMALL_W], f32, kind="ExternalInput").ap()
    pbf_d = nc.dram_tensor("pbf_d", [128, NT * NT * 128], bf16, kind="ExternalInput").ap()
    w3_d = nc.dram_tensor("w3_d", [128, HT * 512], bf16, kind="ExternalInput").ap()
    dw_d = nc.dram_tensor("dw_d", [C, BL + H], bf16, kind="ExternalInput").ap()
    w2t_d = nc.dram_tensor("w2t_d", [H, H], bf16, kind="ExternalInput").ap()
    out_d = nc.dram_tensor("out_d", [128, N2P], f32, kind="ExternalOutput").ap()

    Act = mybir.ActivationFunctionType
    Alu = mybir.AluOpType

    with tile.TileContext(nc) as tc:
        with tc.tile_pool(name="sb", bufs=1) as sb, \
             tc.tile_pool(name="wst", bufs=4) as wst, \
             tc.tile_pool(name="mlp", bufs=1) as mlp, \
             tc.tile_pool(name="ps", bufs=8, space="PSUM") as pspool:
            dw = mlp.tile([C, BL + H], bf16)
            nc.sync.dma_start(out=dw[:], in_=dw_d[:])
            sm = sb.tile([128, SMALL_W], f32)
            nc.sync.dma_start(out=sm[:], in_=small_d[:])
            pbf = sb.tile([128, NT * NT * 128], bf16)

            b1R = sm[:, OFF_B1:OFF_B1 + HT]
            b2R = sm[:, OFF_B2:OFF_B2 + HT]
            b3R = sm[:, OFF_B3:OFF_B3 + CT]
            dT = dw[:, 0:BL]
            w1T = dw[:, BL:BL + H]

            h1 = mlp.tile([128, HT * 128], bf16)  # h1T tiles: [p, m*128+b]
            h2 = mlp.tile([128, HT * 128], bf16)
            w_sb = sb.tile([128, N2P], f32)       # wT tiles: [p, j*128+b]
            wb_sb = sb.tile([128, N2P], bf16)
            q_sb = sb.tile([128, N2P], f32)
            a_sb = sb.tile([128, N2P], f32)
            tb_bufs = [sb.tile([128, N2P], bf16, name=f"tb{i}") for i in range(3)]
            out_sb = sb.tile([128, N2P], f32)

            nc.vector.memset(q_sb[:], 0.0)
            nc.vector.tensor_copy(w_sb[:, CT * 128:],
                                  sm[:, OFF_WP:OFF_WP + (NT - CT) * 128])

            # ---- MLP layer 1: h1T[m] = prelu(W1T[:,m].T @ dT + b1, 0.1) ----
            for m in range(HT):
                ps_t = pspool.tile([128, 128], f32, tag="ps", name="ps_t")
                nc.tensor.matmul(ps_t[:], w1T[:, m * 128:(m + 1) * 128], dT,
                                 start=True, stop=True)
                nc.scalar.activation(h1[:, m * 128:(m + 1) * 128], ps_t[:],
                                     Act.Prelu, bias=b1R[:, m:m + 1], alpha=0.1)

            # ---- MLP layer 2: stream W2T; h2T[m] = prelu(sum_k W2T[k,m].T @ h1T[k] + b2) ----
            mark_mid = None
            for mc in range(N_MC):
                ps_list = [pspool.tile([128, 128], f32, tag="ps", name="ps_t")
                           for _ in range(MC_W)]
                for k in range(HT):
                    w2blk = wst.tile([128, MC_W * 128], bf16, name="w2blk")
                    nc.sync.dma_start(
                        out=w2blk[:],
                        in_=w2t_d[k * 128:(k + 1) * 128,
                                  mc * MC_W * 128:(mc + 1) * MC_W * 128])
                    for mi in range(MC_W):
                        mm = nc.tensor.matmul(ps_list[mi][:],
                                         w2blk[:, mi * 128:(mi + 1) * 128],
                                         h1[:, k * 128:(k + 1) * 128],
                                         start=(k == 0), stop=(k == HT - 1))
                        if mc == 2 and k == 0 and mi == 0:
                            mark_mid = mm.ins
                for mi in range(MC_W):
                    m = mc * MC_W + mi
                    nc.scalar.activation(h2[:, m * 128:(m + 1) * 128], ps_list[mi][:],
                                         Act.Prelu, bias=b2R[:, m:m + 1], alpha=0.1)

            # ---- cost layer: w tiles 0..3 = sum_k W3p.T[k] @ h2T[k] + b3 (w3 streamed) ----
            ps_cost = [pspool.tile([128, 128], f32, tag="ps", name="ps_t")
                       for _ in range(CT)]
            for k in range(HT):
                w3blk = wst.tile([128, 512], bf16, name="w3blk", bufs=3)
                nc.sync.dma_start(out=w3blk[:], in_=w3_d[:, k * 512:(k + 1) * 512])
                for m in range(CT):
                    nc.tensor.matmul(ps_cost[m][:],
                                     w3blk[:, m * 128:(m + 1) * 128],
                                     h2[:, k * 128:(k + 1) * 128],
                                     start=(k == 0), stop=(k == HT - 1))
            for m in range(CT):
                nc.scalar.activation(w_sb[:, m * 128:(m + 1) * 128], ps_cost[m][:],
                                     Act.Identity, bias=b3R[:, m:m + 1])
            nc.vector.tensor_copy(wb_sb[:], w_sb[:])

            pbf_dma = nc.sync.dma_start(out=pbf[:], in_=pbf_d[:])
            add_dep_helper(pbf_dma.ins, mark_mid, sync=True,
                           reason="defer bf16 P load past MLP DMA crunch")

            # ---- ADMM iterations: all bf16 operands, fp32 PSUM accumulate ----
            for it in range(WARM):
                last = (it == WARM - 1)
                cur = wb_sb if it == 0 else tb_bufs[(it - 1) % 3]
                for j in range(NT):
                    ps_t = pspool.tile([128, 128], f32, tag="ps", name="ps_t")
                    for k in range(NT):
                        nc.tensor.matmul(ps_t[:],
                                         pbf[:, (k * NT + j) * 128:(k * NT + j + 1) * 128],
                                         cur[:, k * 128:(k + 1) * 128],
                                         start=(k == 0), stop=(k == NT - 1))
                    jj = slice(j * 128, (j + 1) * 128)
                    if last:
                        nc.scalar.activation(out_sb[:, jj], ps_t[:], Act.Copy)
                        nc.sync.dma_start(out=out_d[:, jj], in_=out_sb[:, jj])
                    else:
                        # q = min(q, 0) + x
                        nc.vector.scalar_tensor_tensor(
                            out=q_sb[:, jj], in0=q_sb[:, jj], scalar=0.0,
                            in1=ps_t[:], op0=Alu.min, op1=Alu.add)
                        nc.scalar.activation(a_sb[:, jj], q_sb[:, jj], Act.Abs)
                        nc.vector.tensor_tensor(out=tb_bufs[it % 3][:, jj],
                                                in0=a_sb[:, jj], in1=w_sb[:, jj],
                                                op=Alu.add)

    nc.compile()
    return nc


def kernel(d, W1, b1, W2, b2, W3, b3, weights_mat, capacities):
    from concourse.bass_utils import run_bass_kernel_spmd
    import ml_dtypes

    d = np.asarray(d, np.float32)
    small, PbigBF, w3PM, W1T, W2T = _host_precompute(
        np.asarray(W1, np.float32), np.asarray(b1, np.float32),
        np.asarray(W2, np.float32), np.asarray(b2, np.float32),
        np.asarray(W3, np.float32), np.asarray(b3, np.float32),
        np.asarray(weights_mat, np.float32), np.asarray(capacities, np.float32))

    if "nc" not in _CACHE:
        _CACHE["nc"] = _build_nc()
    nc = _CACHE["nc"]

    in_maps = []
    for i in range(NCORES):
        dTc = np.ascontiguousarray(d[i * BL:(i + 1) * BL].T).astype(ml_dtypes.bfloat16)
        dwc = np.concatenate([dTc, W1T], axis=1)                  # [32, 128+3200]
        in_maps.append({"small_d": small, "pbf_d": PbigBF,
                        "w3_d": w3PM, "dw_d": dwc, "w2t_d": W2T})

    trace = bool(int(os.environ.get("KNAP_TRACE", "0")))
    res = run_bass_kernel_spmd(nc, in_maps, core_ids=list(range(NCORES)),
                               trace=trace)
    if trace:
        _CACHE["exec_time_ns"] = res.exec_time_ns
        _CACHE["trace"] = res.instructions_and_trace

    out = np.empty((B, N2), np.float32)
    for i in range(NCORES):
        arr = res.results[i]["out_d"]                              # [128, 1152]
        xc = arr.reshape(128, NT, 128).transpose(2, 1, 0).reshape(BL, N2P)
        out[i * BL:(i + 1) * BL] = xc[:, :N2]
    return out


# revision 8
# speedup vs baseline: 3.3532x; 1.4506x over previous
"""TRN2 Bass kernel for nn_Cvx_KnapsackNet (MLP + truncated ADMM projection QP).

Math: the reference ADMM iteration
    v   = w + rho*(z - u)
    rhs = v @ A.T - (1+rho)*b
    y   = cho_solve(A A^T, rhs.T).T
    x   = (v - y @ A) / (1+rho)
    z   = max(x+u, 0);  u = u + x - z
collapses algebraically. With P' = (I - A^T (A A^T)^{-1} A)/(1+rho),
c = b @ (A A^T)^{-1} A, and state q_k = x_k + u_{k-1}:
    t_k     = w + |q_k|          (t_0 = w)
    x_k     = t_k @ P' + c
    q_{k+1} = x_k + min(q_k, 0)
One [B,1152]x[1152,1152] matmul plus two fused elementwise ops per
iteration. c is folded into the matmul via an extra "ones" row (row
1030 of the padded state is held at 1; row 1030 of P' holds c).

The map contracts at ~0.64/iter; the harness gate is rel_err < 2e-2,
so a fully-bf16 pipeline (MLP weights/activations and ADMM operands in
bf16, fp32 PSUM accumulate) with 16 iterations reaches ~1.9e-3 —
dominated by the bf16 MLP cost-vector error, with a 10x margin.
fp32 matmuls run at half rate with unhidden LDWEIGHTS (~4x slower in
practice), so everything streams through the PE in bf16.

Sharding: pure data parallel, batch 1024 -> 128 rows per NeuronCore.
On-chip layout is transposed ([n2p=1152 rows, 128 batch cols], 9 tiles
of 128 partitions) so the matmul contraction runs over partitions.
"""
import sys
sys.path.insert(0, '/opt/trn_rl_repo')
import os
import numpy as np

B, C, H, R, K = 1024, 32, 3200, 500, 30
RHO = 1.0
N1 = K + R              # 530
N2 = R + K + R          # 1030
N2P = 1152              # 9 * 128
NT = N2P // 128         # 9 state tiles
BIAS_ROW = N2           # 1030
NCORES = 8
BL = B // NCORES        # 128 batch rows per core
HT = H // 128           # 25 hidden tiles
WARM = int(os.environ.get("KNAP_WARM", "16"))
MC_W = 5                # m-tiles per W2 chunk
N_MC = HT // MC_W       # 5 chunks
CT = 512 // 128         # 4 cost tiles (500 padded to 512)

_CACHE = {}


def _host_precompute(W1, b1, W2, b2, W3, b3, weights_mat, capacities):
    """float64 host math -> packed bf16/fp32 device constants."""
    import ml_dtypes
    bf = ml_dtypes.bfloat16
    wm = weights_mat.astype(np.float64)
    cap = capacities.astype(np.float64)
    A = np.zeros((N1, N2), np.float64)
    A[:K, :R] = wm
    A[:K, R:R + K] = np.eye(K)
    A[K:, :R] = np.eye(R)
    A[K:, R + K:] = np.eye(R)
    b = np.concatenate([cap, np.ones(R)])
    M = np.linalg.inv(A @ A.T)
    P = (np.eye(N2) - A.T @ M @ A) / (1.0 + RHO)
    c = b @ M @ A
    Pbig = np.zeros((N2P, N2P), np.float32)
    Pbig[:N2, :N2] = P.astype(np.float32)
    Pbig[BIAS_ROW, :N2] = c.astype(np.float32)
    # partition-major blocked: PbigPM[p, (k*NT+j)*128 + f] = Pbig[k*128+p, j*128+f]
    PbigPM = np.ascontiguousarray(
        Pbig.reshape(NT, 128, NT, 128).transpose(1, 0, 2, 3).reshape(128, NT * NT * 128))
    PbigBF = PbigPM.astype(bf)

    W3p = np.zeros((512, H), np.float32)
    W3p[:R] = W3
    # w3PM[p, k*512 + f] = W3p.T[k*128+p, f]
    w3PM = np.ascontiguousarray(
        W3p.T.reshape(HT, 128, 512).transpose(1, 0, 2).reshape(128, HT * 512)).astype(bf)

    # W2 pre-blocked for big contiguous stripe DMAs (32KB/partition lines):
    # W2P[p, ((mc*HT + k)*MC_W + mi)*128 + f] = W2.T[k*128+p, mc*640+mi*128+f]
    W2P = np.ascontiguousarray(
        W2.T.reshape(HT, 128, N_MC, MC_W, 128).transpose(1, 2, 0, 3, 4)
        .reshape(128, H * HT)).astype(bf)

    b1R = np.ascontiguousarray(b1.reshape(HT, 128).T)       # [128, 25]
    b2R = np.ascontiguousarray(b2.reshape(HT, 128).T)       # [128, 25]
    b3p = np.zeros(512, np.float32)
    b3p[:R] = b3
    b3R = np.ascontiguousarray(b3p.reshape(CT, 128).T)      # [128, 4]
    # padding tiles 4..8 of w (zeros; bias-row 1030 -> tile 8, partition 6 = 1)
    wpad = np.zeros((128, (NT - CT) * 128), np.float32)
    wpad[BIAS_ROW - 8 * 128, (8 - CT) * 128:(9 - CT) * 128] = 1.0

    small = np.concatenate([b1R, b2R, b3R, wpad], axis=1).astype(np.float32)
    W1T = np.ascontiguousarray(W1.T).astype(bf)             # [32, 3200]
    return small, PbigBF, w3PM, W1T, W2P


def _build_nc():
    import concourse.bacc as bacc
    import concourse.mybir as mybir
    from concourse import tile
    from concourse.tile_rust import add_dep_helper

    f32 = mybir.dt.float32
    bf16 = mybir.dt.bfloat16
    SMALL_W = HT + HT + CT + (NT - CT) * 128
    OFF_B1 = 0
    OFF_B2 = OFF_B1 + HT
    OFF_B3 = OFF_B2 + HT
    OFF_WP = OFF_B3 + CT

    nc = bacc.Bacc("TRN2", target_bir_lowering=False, debug=False, num_devices=NCORES)
    small_d = nc.dram_tensor("small_d", [128, SMALL_W], f32, kind="ExternalInput").ap()
    pbf_d = nc.dram_tensor("pbf_d", [128, NT * NT * 128], bf16, kind="ExternalInput").ap()
    w3_d = nc.dram_tensor("w3_d", [128, HT * 512], bf16, kind="ExternalInput").ap()
    dw_d = nc.dram_tensor("dw_d", [C, BL + H], bf16, kind="ExternalInput").ap()
    w2_d = nc.dram_tensor("w2_d", [128, H * HT], bf16, kind="ExternalInput").ap()
    out_d = nc.dram_tensor("out_d", [128, N2P], f32, kind="ExternalOutput").ap()

    Act = mybir.ActivationFunctionType
    Alu = mybir.AluOpType
    SW = HT * MC_W * 128    # stripe width: 16000 cols

    with tile.TileContext(nc) as tc:
        with tc.tile_pool(name="sb", bufs=1) as sb, \
             tc.tile_pool(name="wst", bufs=2) as wst, \
             tc.tile_pool(name="mlp", bufs=1) as mlp, \
             tc.tile_pool(name="ps", bufs=8, space="PSUM") as pspool:
            # constants ride the Act HWDGE ring; W2 stripes own the SP ring
            dw = mlp.tile([C, BL + H], bf16)
            nc.scalar.dma_start(out=dw[:], in_=dw_d[:])
            sm = sb.tile([128, SMALL_W], f32)
            nc.scalar.dma_start(out=sm[:], in_=small_d[:])
            pbf = sb.tile([128, NT * NT * 128], bf16)
            w3t = mlp.tile([128, HT * 512], bf16)

            b1R = sm[:, OFF_B1:OFF_B1 + HT]
            b2R = sm[:, OFF_B2:OFF_B2 + HT]
            b3R = sm[:, OFF_B3:OFF_B3 + CT]
            dT = dw[:, 0:BL]
            w1T = dw[:, BL:BL + H]

            h1 = mlp.tile([128, HT * 128], bf16)  # h1T tiles: [p, m*128+b]
            h2 = mlp.tile([128, HT * 128], bf16)
            w_sb = sb.tile([128, N2P], f32)       # wT tiles: [p, j*128+b]
            wb_sb = sb.tile([128, N2P], bf16)
            q_sb = sb.tile([128, N2P], f32)
            a_sb = sb.tile([128, N2P], f32)
            tb_bufs = [sb.tile([128, N2P], bf16, name=f"tb{i}") for i in range(3)]
            out_sb = sb.tile([128, N2P], f32)

            nc.vector.memset(q_sb[:], 0.0)
            nc.vector.tensor_copy(w_sb[:, CT * 128:],
                                  sm[:, OFF_WP:OFF_WP + (NT - CT) * 128])

            # ---- MLP layer 1: h1T[m] = prelu(W1T[:,m].T @ dT + b1, 0.1) ----
            for m in range(HT):
                ps_t = pspool.tile([128, 128], f32, tag="ps", name="ps_t")
                nc.tensor.matmul(ps_t[:], w1T[:, m * 128:(m + 1) * 128], dT,
                                 start=True, stop=True)
                nc.scalar.activation(h1[:, m * 128:(m + 1) * 128], ps_t[:],
                                     Act.Prelu, bias=b1R[:, m:m + 1], alpha=0.1)

            # ---- MLP layer 2: stream W2 stripes; h2T[m] = prelu(sum_k W2T[k,m].T @ h1T[k] + b2) ----
            mark_w3 = None
            mark_pbf = None
            for mc in range(N_MC):
                stripe = wst.tile([128, SW], bf16, name="w2s")
                nc.sync.dma_start(out=stripe[:], in_=w2_d[:, mc * SW:(mc + 1) * SW])
                ps_list = [pspool.tile([128, 128], f32, tag="ps", name="ps_t")
                           for _ in range(MC_W)]
                for k in range(HT):
                    for mi in range(MC_W):
                        mm = nc.tensor.matmul(ps_list[mi][:],
                                         stripe[:, (k * MC_W + mi) * 128:(k * MC_W + mi + 1) * 128],
                                         h1[:, k * 128:(k + 1) * 128],
                                         start=(k == 0), stop=(k == HT - 1))
                        if mc == 2 and k == 0 and mi == 0:
                            mark_w3 = mm.ins
                        if mc == 3 and k == 0 and mi == 0:
                            mark_pbf = mm.ins
                for mi in range(MC_W):
                    m = mc * MC_W + mi
                    nc.scalar.activation(h2[:, m * 128:(m + 1) * 128], ps_list[mi][:],
                                         Act.Prelu, bias=b2R[:, m:m + 1], alpha=0.1)

            # W3 + P loads on the Act ring, deferred past the W2 stream crunch
            w3_dma = nc.scalar.dma_start(out=w3t[:], in_=w3_d[:])
            add_dep_helper(w3_dma.ins, mark_w3, sync=True,
                           reason="defer w3 load past W2 stripe crunch")
            pbf_dma = nc.scalar.dma_start(out=pbf[:], in_=pbf_d[:])
            add_dep_helper(pbf_dma.ins, mark_pbf, sync=True,
                           reason="defer bf16 P load past W2 stripe crunch")

            # ---- cost layer: w tiles 0..3 = sum_k W3p.T[k] @ h2T[k] + b3 ----
            ps_cost = [pspool.tile([128, 128], f32, tag="ps", name="ps_t")
                       for _ in range(CT)]
            for k in range(HT):
                for m in range(CT):
                    nc.tensor.matmul(ps_cost[m][:],
                                     w3t[:, k * 512 + m * 128:k * 512 + (m + 1) * 128],
                                     h2[:, k * 128:(k + 1) * 128],
                                     start=(k == 0), stop=(k == HT - 1))
            for m in range(CT):
                nc.scalar.activation(w_sb[:, m * 128:(m + 1) * 128], ps_cost[m][:],
                                     Act.Identity, bias=b3R[:, m:m + 1])
            nc.vector.tensor_copy(wb_sb[:], w_sb[:])

            # ---- ADMM iterations: all bf16 operands, fp32 PSUM accumulate ----
            for it in range(WARM):
                last = (it == WARM - 1)
                cur = wb_sb if it == 0 else tb_bufs[(it - 1) % 3]
                for j in range(NT):
                    ps_t = pspool.tile([128, 128], f32, tag="ps", name="ps_t")
                    for k in range(NT):
                        nc.tensor.matmul(ps_t[:],
                                         pbf[:, (k * NT + j) * 128:(k * NT + j + 1) * 128],
                                         cur[:, k * 128:(k + 1) * 128],
                                         start=(k == 0), stop=(k == NT - 1))
                    jj = slice(j * 128, (j + 1) * 128)
                    if last:
                        nc.scalar.activation(out_sb[:, jj], ps_t[:], Act.Copy)
                        nc.sync.dma_start(out=out_d[:, jj], in_=out_sb[:, jj])
                    else:
                        # q = min(q, 0) + x
                        nc.vector.scalar_tensor_tensor(
                            out=q_sb[:, jj], in0=q_sb[:, jj], scalar=0.0,
                            in1=ps_t[:], op0=Alu.min, op1=Alu.add)
                        nc.scalar.activation(a_sb[:, jj], q_sb[:, jj], Act.Abs)
                        nc.vector.tensor_tensor(out=tb_bufs[it % 3][:, jj],
                                                in0=a_sb[:, jj], in1=w_sb[:, jj],
                                                op=Alu.add)

    nc.compile()
    return nc


def kernel(d, W1, b1, W2, b2, W3, b3, weights_mat, capacities):
    from concourse.bass_utils import run_bass_kernel_spmd
    import ml_dtypes

    d = np.asarray(d, np.float32)
    small, PbigBF, w3PM, W1T, W2P = _host_precompute(
        np.asarray(W1, np.float32), np.asarray(b1, np.float32),
        np.asarray(W2, np.float32), np.asarray(b2, np.float32),
        np.asarray(W3, np.float32), np.asarray(b3, np.float32),
        np.asarray(weights_mat, np.float32), np.asarray(capacities, np.float32))

    if "nc" not in _CACHE:
        _CACHE["nc"] = _build_nc()
    nc = _CACHE["nc"]

    in_maps = []
    for i in range(NCORES):
        dTc = np.ascontiguousarray(d[i * BL:(i + 1) * BL].T).astype(ml_dtypes.bfloat16)
        dwc = np.concatenate([dTc, W1T], axis=1)                  # [32, 128+3200]
        in_maps.append({"small_d": small, "pbf_d": PbigBF,
                        "w3_d": w3PM, "dw_d": dwc, "w2_d": W2P})

    trace = bool(int(os.environ.get("KNAP_TRACE", "0")))
    res = run_bass_kernel_spmd(nc, in_maps, core_ids=list(range(NCORES)),
                               trace=trace)
    if trace:
        _CACHE["exec_time_ns"] = res.exec_time_ns
        _CACHE["trace"] = res.instructions_and_trace

    out = np.empty((B, N2), np.float32)
    for i in range(NCORES):
        arr = res.results[i]["out_d"]                              # [128, 1152]
        xc = arr.reshape(128, NT, 128).transpose(2, 1, 0).reshape(BL, N2P)
        out[i * BL:(i + 1) * BL] = xc[:, :N2]
    return out


# revision 9
# speedup vs baseline: 3.9756x; 1.1856x over previous
"""TRN2 Bass kernel for nn_Cvx_KnapsackNet (MLP + truncated ADMM projection QP).

Math: the reference ADMM iteration
    v   = w + rho*(z - u)
    rhs = v @ A.T - (1+rho)*b
    y   = cho_solve(A A^T, rhs.T).T
    x   = (v - y @ A) / (1+rho)
    z   = max(x+u, 0);  u = u + x - z
collapses algebraically. With P' = (I - A^T (A A^T)^{-1} A)/(1+rho),
c = b @ (A A^T)^{-1} A, and state q_k = x_k + u_{k-1}:
    t_k     = w + |q_k|          (t_0 = w)
    x_k     = t_k @ P' + c
    q_{k+1} = x_k + min(q_k, 0)
One [B,1152]x[1152,1152] matmul plus two fused elementwise ops per
iteration. c is folded into the matmul via an extra "ones" row (row
1030 of the padded state is held at 1; row 1030 of P' holds c).

The map contracts at ~0.64/iter; the harness gate is rel_err < 2e-2,
so a fully-bf16 pipeline (MLP weights/activations and ADMM operands in
bf16, fp32 PSUM accumulate) with 16 iterations reaches ~1.9e-3 —
dominated by the bf16 MLP cost-vector error, with a 10x margin.
fp32 matmuls run at half rate with unhidden LDWEIGHTS (~4x slower in
practice), so everything streams through the PE in bf16.

Sharding: pure data parallel, batch 1024 -> 128 rows per NeuronCore.
On-chip layout is transposed ([n2p=1152 rows, 128 batch cols], 9 tiles
of 128 partitions) so the matmul contraction runs over partitions.
"""
import sys
sys.path.insert(0, '/opt/trn_rl_repo')
import os
import numpy as np

B, C, H, R, K = 1024, 32, 3200, 500, 30
RHO = 1.0
N1 = K + R              # 530
N2 = R + K + R          # 1030
N2P = 1152              # 9 * 128
NT = N2P // 128         # 9 state tiles
BIAS_ROW = N2           # 1030
NCORES = 8
BL = B // NCORES        # 128 batch rows per core
HT = H // 128           # 25 hidden tiles
WARM = int(os.environ.get("KNAP_WARM", "12"))
MC_W = 5                # m-tiles per W2 chunk
N_MC = HT // MC_W       # 5 chunks
CT = 512 // 128         # 4 cost tiles (500 padded to 512)

_CACHE = {}


def _host_precompute(W1, b1, W2, b2, W3, b3, weights_mat, capacities):
    """float64 host math -> packed bf16/fp32 device constants."""
    import ml_dtypes
    bf = ml_dtypes.float16
    wm = weights_mat.astype(np.float64)
    cap = capacities.astype(np.float64)
    A = np.zeros((N1, N2), np.float64)
    A[:K, :R] = wm
    A[:K, R:R + K] = np.eye(K)
    A[K:, :R] = np.eye(R)
    A[K:, R + K:] = np.eye(R)
    b = np.concatenate([cap, np.ones(R)])
    M = np.linalg.inv(A @ A.T)
    P = (np.eye(N2) - A.T @ M @ A) / (1.0 + RHO)
    c = b @ M @ A
    Pbig = np.zeros((N2P, N2P), np.float32)
    Pbig[:N2, :N2] = P.astype(np.float32)
    Pbig[BIAS_ROW, :N2] = c.astype(np.float32)
    # partition-major blocked: PbigPM[p, (k*NT+j)*128 + f] = Pbig[k*128+p, j*128+f]
    PbigPM = np.ascontiguousarray(
        Pbig.reshape(NT, 128, NT, 128).transpose(1, 0, 2, 3).reshape(128, NT * NT * 128))
    PbigBF = PbigPM.astype(bf)

    W3p = np.zeros((512, H), np.float32)
    W3p[:R] = W3
    # w3PM[p, k*512 + f] = W3p.T[k*128+p, f]
    w3PM = np.ascontiguousarray(
        W3p.T.reshape(HT, 128, 512).transpose(1, 0, 2).reshape(128, HT * 512)).astype(bf)

    # W2 pre-blocked for big contiguous stripe DMAs (32KB/partition lines):
    # W2P[p, ((mc*HT + k)*MC_W + mi)*128 + f] = W2.T[k*128+p, mc*640+mi*128+f]
    W2P = np.ascontiguousarray(
        W2.T.reshape(HT, 128, N_MC, MC_W, 128).transpose(1, 2, 0, 3, 4)
        .reshape(128, H * HT)).astype(bf)

    b1R = np.ascontiguousarray(b1.reshape(HT, 128).T)       # [128, 25]
    b2R = np.ascontiguousarray(b2.reshape(HT, 128).T)       # [128, 25]
    b3p = np.zeros(512, np.float32)
    b3p[:R] = b3
    b3R = np.ascontiguousarray(b3p.reshape(CT, 128).T)      # [128, 4]
    # padding tiles 4..8 of w (zeros; bias-row 1030 -> tile 8, partition 6 = 1)
    wpad = np.zeros((128, (NT - CT) * 128), np.float32)
    wpad[BIAS_ROW - 8 * 128, (8 - CT) * 128:(9 - CT) * 128] = 1.0

    small = np.concatenate([b1R, b2R, b3R, wpad], axis=1).astype(np.float32)
    W1T = np.ascontiguousarray(W1.T).astype(bf)             # [32, 3200]
    return small, PbigBF, w3PM, W1T, W2P


def _build_nc():
    import concourse.bacc as bacc
    import concourse.mybir as mybir
    from concourse import tile
    from concourse.tile_rust import add_dep_helper

    f32 = mybir.dt.float32
    bf16 = mybir.dt.float16
    SMALL_W = HT + HT + CT + (NT - CT) * 128
    OFF_B1 = 0
    OFF_B2 = OFF_B1 + HT
    OFF_B3 = OFF_B2 + HT
    OFF_WP = OFF_B3 + CT

    nc = bacc.Bacc("TRN2", target_bir_lowering=False, debug=False, num_devices=NCORES)
    small_d = nc.dram_tensor("small_d", [128, SMALL_W], f32, kind="ExternalInput").ap()
    pbf_d = nc.dram_tensor("pbf_d", [128, NT * NT * 128], bf16, kind="ExternalInput").ap()
    w3_d = nc.dram_tensor("w3_d", [128, HT * 512], bf16, kind="ExternalInput").ap()
    dw_d = nc.dram_tensor("dw_d", [C, BL + H], bf16, kind="ExternalInput").ap()
    w2_d = nc.dram_tensor("w2_d", [128, H * HT], bf16, kind="ExternalInput").ap()
    out_d = nc.dram_tensor("out_d", [128, N2P], f32, kind="ExternalOutput").ap()

    Act = mybir.ActivationFunctionType
    Alu = mybir.AluOpType
    SW = HT * MC_W * 128    # stripe width: 16000 cols

    with tile.TileContext(nc) as tc:
        with tc.tile_pool(name="sb", bufs=1) as sb, \
             tc.tile_pool(name="wst", bufs=3) as wst, \
             tc.tile_pool(name="mlp", bufs=1) as mlp, \
             tc.tile_pool(name="ps", bufs=8, space="PSUM") as pspool:
            # constants ride the Act HWDGE ring; W2 stripes own the SP ring
            dw = mlp.tile([C, BL + H], bf16)
            nc.scalar.dma_start(out=dw[:], in_=dw_d[:])
            sm = sb.tile([128, SMALL_W], f32)
            nc.scalar.dma_start(out=sm[:], in_=small_d[:])
            pbf = sb.tile([128, NT * NT * 128], bf16)
            w3t = mlp.tile([128, HT * 512], bf16)

            b1R = sm[:, OFF_B1:OFF_B1 + HT]
            b2R = sm[:, OFF_B2:OFF_B2 + HT]
            b3R = sm[:, OFF_B3:OFF_B3 + CT]
            dT = dw[:, 0:BL]
            w1T = dw[:, BL:BL + H]

            h1 = mlp.tile([128, HT * 128], bf16)  # h1T tiles: [p, m*128+b]
            h2 = mlp.tile([128, HT * 128], bf16)
            w_sb = sb.tile([128, N2P], f32)       # wT tiles: [p, j*128+b]
            wb_sb = sb.tile([128, N2P], bf16)
            q_sb = sb.tile([128, N2P], f32)
            a_sb = sb.tile([128, N2P], f32)
            tb_bufs = [sb.tile([128, N2P], bf16, name=f"tb{i}") for i in range(3)]
            out_sb = sb.tile([128, N2P], f32)

            nc.vector.memset(q_sb[:], 0.0)
            nc.vector.tensor_copy(w_sb[:, CT * 128:],
                                  sm[:, OFF_WP:OFF_WP + (NT - CT) * 128])

            # ---- MLP layer 1: h1T[m] = prelu(W1T[:,m].T @ dT + b1, 0.1) ----
            for m in range(HT):
                ps_t = pspool.tile([128, 128], f32, tag="ps", name="ps_t")
                nc.tensor.matmul(ps_t[:], w1T[:, m * 128:(m + 1) * 128], dT,
                                 start=True, stop=True)
                nc.scalar.activation(h1[:, m * 128:(m + 1) * 128], ps_t[:],
                                     Act.Prelu, bias=b1R[:, m:m + 1], alpha=0.1)

            # ---- MLP layer 2: stream W2 stripes; h2T[m] = prelu(sum_k W2T[k,m].T @ h1T[k] + b2) ----
            mark_w3 = None
            mark_pbf = None
            for mc in range(N_MC):
                stripe = wst.tile([128, SW], bf16, name="w2s")
                nc.sync.dma_start(out=stripe[:], in_=w2_d[:, mc * SW:(mc + 1) * SW])
                ps_list = [pspool.tile([128, 128], f32, tag="ps", name="ps_t")
                           for _ in range(MC_W)]
                for k in range(HT):
                    for mi in range(MC_W):
                        mm = nc.tensor.matmul(ps_list[mi][:],
                                         stripe[:, (k * MC_W + mi) * 128:(k * MC_W + mi + 1) * 128],
                                         h1[:, k * 128:(k + 1) * 128],
                                         start=(k == 0), stop=(k == HT - 1))
                        if mc == 2 and k == 0 and mi == 0:
                            mark_w3 = mm.ins
                        if mc == 3 and k == 0 and mi == 0:
                            mark_pbf = mm.ins
                for mi in range(MC_W):
                    m = mc * MC_W + mi
                    nc.scalar.activation(h2[:, m * 128:(m + 1) * 128], ps_list[mi][:],
                                         Act.Prelu, bias=b2R[:, m:m + 1], alpha=0.1)

            # W3 + P loads on the Act ring, deferred past the W2 stream crunch
            w3_dma = nc.scalar.dma_start(out=w3t[:], in_=w3_d[:])
            add_dep_helper(w3_dma.ins, mark_w3, sync=True,
                           reason="defer w3 load past W2 stripe crunch")
            pbf_dma = nc.scalar.dma_start(out=pbf[:], in_=pbf_d[:])
            add_dep_helper(pbf_dma.ins, mark_pbf, sync=True,
                           reason="defer bf16 P load past W2 stripe crunch")

            # ---- cost layer: w tiles 0..3 = sum_k W3p.T[k] @ h2T[k] + b3 ----
            ps_cost = [pspool.tile([128, 128], f32, tag="ps", name="ps_t")
                       for _ in range(CT)]
            for k in range(HT):
                for m in range(CT):
                    nc.tensor.matmul(ps_cost[m][:],
                                     w3t[:, k * 512 + m * 128:k * 512 + (m + 1) * 128],
                                     h2[:, k * 128:(k + 1) * 128],
                                     start=(k == 0), stop=(k == HT - 1))
            for m in range(CT):
                nc.scalar.activation(w_sb[:, m * 128:(m + 1) * 128], ps_cost[m][:],
                                     Act.Identity, bias=b3R[:, m:m + 1])
            nc.vector.tensor_copy(wb_sb[:], w_sb[:])

            # ---- ADMM iterations: all bf16 operands, fp32 PSUM accumulate ----
            for it in range(WARM):
                last = (it == WARM - 1)
                cur = wb_sb if it == 0 else tb_bufs[(it - 1) % 3]
                for j in range(NT):
                    ps_t = pspool.tile([128, 128], f32, tag="ps", name="ps_t")
                    for k in range(NT):
                        nc.tensor.matmul(ps_t[:],
                                         pbf[:, (k * NT + j) * 128:(k * NT + j + 1) * 128],
                                         cur[:, k * 128:(k + 1) * 128],
                                         start=(k == 0), stop=(k == NT - 1))
                    jj = slice(j * 128, (j + 1) * 128)
                    if last:
                        nc.scalar.activation(out_sb[:, jj], ps_t[:], Act.Copy)
                        nc.sync.dma_start(out=out_d[:, jj], in_=out_sb[:, jj])
                    else:
                        # q = min(q, 0) + x
                        nc.vector.scalar_tensor_tensor(
                            out=q_sb[:, jj], in0=q_sb[:, jj], scalar=0.0,
                            in1=ps_t[:], op0=Alu.min, op1=Alu.add)
                        nc.scalar.activation(a_sb[:, jj], q_sb[:, jj], Act.Abs)
                        nc.vector.tensor_tensor(out=tb_bufs[it % 3][:, jj],
                                                in0=a_sb[:, jj], in1=w_sb[:, jj],
                                                op=Alu.add)

    nc.compile()
    return nc


def kernel(d, W1, b1, W2, b2, W3, b3, weights_mat, capacities):
    from concourse.bass_utils import run_bass_kernel_spmd
    import ml_dtypes

    d = np.asarray(d, np.float32)
    small, PbigBF, w3PM, W1T, W2P = _host_precompute(
        np.asarray(W1, np.float32), np.asarray(b1, np.float32),
        np.asarray(W2, np.float32), np.asarray(b2, np.float32),
        np.asarray(W3, np.float32), np.asarray(b3, np.float32),
        np.asarray(weights_mat, np.float32), np.asarray(capacities, np.float32))

    if "nc" not in _CACHE:
        _CACHE["nc"] = _build_nc()
    nc = _CACHE["nc"]

    in_maps = []
    for i in range(NCORES):
        dTc = np.ascontiguousarray(d[i * BL:(i + 1) * BL].T).astype(ml_dtypes.float16)
        dwc = np.concatenate([dTc, W1T], axis=1)                  # [32, 128+3200]
        in_maps.append({"small_d": small, "pbf_d": PbigBF,
                        "w3_d": w3PM, "dw_d": dwc, "w2_d": W2P})

    trace = bool(int(os.environ.get("KNAP_TRACE", "0")))
    res = run_bass_kernel_spmd(nc, in_maps, core_ids=list(range(NCORES)),
                               trace=trace)
    if trace:
        _CACHE["exec_time_ns"] = res.exec_time_ns
        _CACHE["trace"] = res.instructions_and_trace

    out = np.empty((B, N2), np.float32)
    for i in range(NCORES):
        arr = res.results[i]["out_d"]                              # [128, 1152]
        xc = arr.reshape(128, NT, 128).transpose(2, 1, 0).reshape(BL, N2P)
        out[i * BL:(i + 1) * BL] = xc[:, :N2]
    return out
